# revision 1
# baseline (speedup 1.0000x reference)
"""Trainium2 Bass kernel for nn_AttenuationToRainRate (dense_mlp).

Data-parallel over 8 NeuronCores: each core processes B/8 = 32768 samples.

Math (per sample b):
  style = mw3 @ relu(mw2 @ relu(mw1 @ md + mb1) + mb2) + mb3      [1024]
  layer L (L=0..3): scale_c = style[256L+2c], bias_c = style[256L+2c+1]
  y(1)  = w1 x + b1;  y(L+1) = wL z(L) + bL
  z(L)  = lrelu(scale * (y - mean_c y)/ (std_c y + 1e-6) + bias)   (std ddof=1)
  out   = lrelu(w5 z(4) + b5)

Device layout: channels on SBUF partitions, batch along the free dim in
chunks of 512. Host-side weight preprocessing:
  * trunk weights are column-centered (subtract per-column output-mean), so
    the matmul directly produces y - mean(y): no mean pass on device.
  * mw3 rows are permuted into per-layer scale/bias matrices, so the style
    matmuls produce scale[c,b] / bias[c,b] tiles directly (no deinterleave).
  * variance = ones(128,128)/127 @ yc^2 : one matmul both reduces over the
    partition (channel) axis and broadcasts the result to all partitions.
"""

import os
import sys

import numpy as np

for p in ("/opt/trn_rl_repo", "/root/.axon_site/_ro/trn_rl_repo"):
    if os.path.isdir(p) and p not in sys.path:
        sys.path.insert(0, p)

import concourse.bass as bass
import concourse.bacc as bacc
import concourse.mybir as mybir
from concourse.tile import TileContext
from concourse import bass_utils

B = 262144
MF = 16
C = 128
NCORES = 8
BL = B // NCORES          # 32768 samples per core
CHUNK = 512               # batch columns per matmul (fp32 PSUM bank limit)
SUPER = 2048              # batch columns per DMA staging tile
NSUPER = BL // SUPER      # 8
CPS = SUPER // CHUNK      # 8 chunks per superchunk
F32R = mybir.dt.float32r
F32 = mybir.dt.float32
AF = mybir.ActivationFunctionType
ALU = mybir.AluOpType
EPS = 1e-6
Z_ACT = 2
C_DVE = 1


def _build(emit_style_bias, emit_trunk_bias, b5_val, reps=1):
    """Build the SPMD Bass program (shared by all 8 cores)."""
    nc = bacc.Bacc("TRN2", target_bir_lowering=False, debug=False)

    # ---- DRAM I/O ----
    d_x = nc.dram_tensor("xt", [1, BL], F32R, kind="ExternalInput")
    d_md = nc.dram_tensor("mdt", [MF, BL], F32R, kind="ExternalInput")
    nw = 1857 + (1024 if emit_style_bias else 0) + (512 if emit_trunk_bias else 0)
    d_wp = nc.dram_tensor("wpack", [C, nw], F32R, kind="ExternalInput")
    d_bp = nc.dram_tensor("bpack", [C, 4], F32, kind="ExternalInput")
    d_out = nc.dram_tensor("out", [BL // CHUNK, CHUNK], F32, kind="ExternalOutput")
    d_den = nc.dram_tensor("den", [BL // CHUNK, CHUNK], F32, kind="ExternalOutput")

    from contextlib import ExitStack
    with TileContext(nc) as tc, ExitStack() as es:
        wp = es.enter_context(tc.tile_pool(name="wp", bufs=1))
        iop = es.enter_context(tc.tile_pool(name="iop", bufs=2))
        ewp = es.enter_context(tc.tile_pool(name="ewp", bufs=7))
        stp = es.enter_context(tc.tile_pool(name="stp", bufs=2))
        psA = es.enter_context(tc.tile_pool(name="psA", bufs=8, space="PSUM"))
        psH = psY = psS = psB = psV = psA

        # ---- resident weights (single packed DMA each) ----
        t_wp = wp.tile([C, nw], F32R)
        nc.sync.dma_start(t_wp[:], d_wp[:])
        t_bp = wp.tile([C, 4], F32)
        nc.sync.dma_start(t_bp[:], d_bp[:])
        t_mw1 = t_wp[0:MF, 0:64]
        t_mw2 = t_wp[0:64, 64:192]
        t_sw = t_wp[:, 192:704]
        t_bw = t_wp[:, 704:1216]
        t_wc = t_wp[:, 1216:1600]
        t_ones = t_wp[:, 1600:1728]
        t_w1 = t_wp[0:1, 1728:1856]
        t_w5 = t_wp[:, 1856:1857]
        t_mb1 = t_bp[0:64, 0:1]
        t_mb2 = t_bp[:, 1:2]
        t_b5 = t_bp[0:1, 2:3]
        t_eps = t_bp[:, 3:4]
        t_sb = t_bb = t_bct = t_one_row = None
        o = 1857
        if emit_style_bias or emit_trunk_bias:
            t_one_row = wp.tile([1, CHUNK], F32R)
            nc.vector.memset(t_one_row[:], 1.0)
        if emit_style_bias:
            t_sb = t_wp[0:1, o:o + 512]
            t_bb = t_wp[0:1, o + 512:o + 1024]
            o += 1024
        if emit_trunk_bias:
            t_bct = t_wp[0:1, o:o + 512]

        rep_cm = tc.For_i(0, reps, 1) if reps > 1 else None
        if rep_cm is not None:
            es.enter_context(rep_cm)
        for s in range(NSUPER):
            c0 = s * SUPER
            t_md = iop.tile([MF, SUPER], F32R, tag="md")
            nc.sync.dma_start(t_md[:], d_md[:, c0:c0 + SUPER])
            t_x = iop.tile([1, SUPER], F32R, tag="x")
            nc.sync.dma_start(t_x[:], d_x[:, c0:c0 + SUPER])

            G = 4
            for jg in range(0, CPS, G):
                js = [jg + g for g in range(G)]
                os_ = [j * CHUNK for j in js]
                h1P = [psH.tile([64, CHUNK], F32, tag="ps", name="h1P") for _ in js]
                for g, j in enumerate(js):
                    nc.tensor.matmul(h1P[g][:], t_mw1,
                                     t_md[:, os_[g]:os_[g] + CHUNK],
                                     start=True, stop=True)
                h1S = [ewp.tile([64, CHUNK], F32R, tag="h1S", name="h1S") for _ in js]
                for g in range(G):
                    nc.scalar.activation(h1S[g][:], h1P[g][:], AF.Relu,
                                         bias=t_mb1)
                h2P = [psH.tile([C, CHUNK], F32, tag="ps", name="h2P") for _ in js]
                for g in range(G):
                    nc.tensor.matmul(h2P[g][:], t_mw2, h1S[g][:],
                                     start=True, stop=True)
                h2S = [ewp.tile([C, CHUNK], F32R, tag="h2S", name="h2S") for _ in js]
                for g in range(G):
                    nc.scalar.activation(h2S[g][:], h2P[g][:], AF.Relu,
                                         bias=t_mb2)
                ycP = [psH.tile([C, CHUNK], F32, tag="ps", name="ycP") for _ in js]
                for g in range(G):
                    nc.tensor.matmul(ycP[g][:], t_w1,
                                     t_x[:, os_[g]:os_[g] + CHUNK],
                                     start=True, stop=not emit_trunk_bias)
                    if emit_trunk_bias:
                        nc.tensor.matmul(ycP[g][:], t_bct[:, 0:C],
                                         t_one_row[:], start=False, stop=True)

                zS = [None] * G
                dS = [None] * G
                for L in range(4):
                    w0 = L * C
                    scP = [psH.tile([C, CHUNK], F32, tag="ps", name="scP") for _ in js]
                    biP = [psH.tile([C, CHUNK], F32, tag="ps", name="biP") for _ in js]
                    for g in range(G):
                        nc.tensor.matmul(scP[g][:], t_sw[:, w0:w0 + C],
                                         h2S[g][:], start=True,
                                         stop=not emit_style_bias)
                        nc.tensor.matmul(biP[g][:], t_bw[:, w0:w0 + C],
                                         h2S[g][:], start=True,
                                         stop=not emit_style_bias)
                        if emit_style_bias:
                            nc.tensor.matmul(scP[g][:], t_sb[:, w0:w0 + C],
                                             t_one_row[:], start=False,
                                             stop=True)
                            nc.tensor.matmul(biP[g][:], t_bb[:, w0:w0 + C],
                                             t_one_row[:], start=False,
                                             stop=True)
                    ycS = [ewp.tile([C, CHUNK], F32, tag="ycS", name="ycS") for _ in js]
                    for g in range(G):
                        if (L + g) % 4 < C_DVE:
                            nc.vector.tensor_copy(ycS[g][:], ycP[g][:])
                        else:
                            nc.scalar.activation(ycS[g][:], ycP[g][:], AF.Copy)
                    sqS = [ewp.tile([C, CHUNK], F32R, tag="sq", name="sqS") for _ in js]
                    for g in range(G):
                        nc.gpsimd.tensor_mul(sqS[g][:], ycS[g][:], ycS[g][:])
                    vP = [psH.tile([C, CHUNK], F32, tag="ps", name="vP") for _ in js]
                    for g in range(G):
                        nc.tensor.matmul(vP[g][:], t_ones, sqS[g][:],
                                         start=True, stop=True)
                    sigS = [ewp.tile([C, CHUNK], F32, tag="sig", name="sigS") for _ in js]
                    for g in range(G):
                        nc.scalar.activation(sigS[g][:], vP[g][:], AF.Sqrt)
                    dN = [None] * G
                    for g in range(G):
                        if L == 0:
                            dN[g] = ewp.tile([C, CHUNK], F32, tag="d", name="dN")
                            nc.vector.tensor_scalar_add(dN[g][:], sigS[g][:],
                                                        EPS)
                        else:
                            dN[g] = sigS[g]
                    m1 = [ewp.tile([C, CHUNK], F32, tag="m1", name="m1") for _ in js]
                    for g in range(G):
                        nc.vector.tensor_mul(m1[g][:], scP[g][:], ycS[g][:])
                    m2 = [ewp.tile([C, CHUNK], F32, tag="m2", name="m2") for _ in js]
                    for g in range(G):
                        nc.vector.tensor_mul(m2[g][:], biP[g][:], dN[g][:])
                    uS = [ewp.tile([C, CHUNK], F32, tag="u", name="uS") for _ in js]
                    for g in range(G):
                        nc.gpsimd.tensor_add(uS[g][:], m1[g][:], m2[g][:])
                    for g in range(G):
                        zS[g] = ewp.tile([C, CHUNK], F32R, tag="z", name="zSg")
                        if (L + g) % 4 < Z_ACT:
                            nc.scalar.activation(zS[g][:], uS[g][:], AF.Prelu,
                                                 alpha=0.01)
                        else:
                            nc.vector.scalar_tensor_tensor(
                                zS[g][:], uS[g][:], 0.01, uS[g][:],
                                op0=ALU.mult, op1=ALU.max)
                        dS[g] = dN[g]

                    if L < 3:
                        ycP = [psH.tile([C, CHUNK], F32, tag="ps", name="ycP") for _ in js]
                        for g in range(G):
                            nc.tensor.matmul(ycP[g][:], t_wc[:, w0:w0 + C],
                                             zS[g][:], start=True,
                                             stop=not emit_trunk_bias)
                            if emit_trunk_bias:
                                nc.tensor.matmul(
                                    ycP[g][:],
                                    t_bct[:, (L + 1) * C:(L + 2) * C],
                                    t_one_row[:], start=False, stop=True)

                outP = [psH.tile([1, CHUNK], F32, tag="ps", name="outP") for _ in js]
                for g in range(G):
                    nc.tensor.matmul(outP[g][:], t_w5, zS[g][:],
                                     start=True, stop=True)
                for g, j in enumerate(js):
                    orow = stp.tile([1, CHUNK], F32, tag="orow")
                    if b5_val != 0.0:
                        nrow = stp.tile([1, CHUNK], F32, tag="nrow")
                        nc.vector.scalar_tensor_tensor(
                            nrow[:], dS[g][0:1, :], float(b5_val), outP[g][:],
                            op0=ALU.mult, op1=ALU.add)
                        nc.scalar.activation(orow[:], nrow[:], AF.Prelu,
                                             alpha=0.01)
                    else:
                        nc.scalar.activation(orow[:], outP[g][:], AF.Prelu,
                                             alpha=0.01)
                    nc.sync.dma_start(d_out[s * CPS + j:s * CPS + j + 1, :],
                                      orow[:])
                    nc.sync.dma_start(d_den[s * CPS + j:s * CPS + j + 1, :],
                                      dS[g][0:1, :])

    nc.compile()
    return nc


def _prep(x, metadata, mw1, mb1, mw2, mb2, mw3, mb3,
          w1, b1, w2, b2, w3, b3, w4, b4, w5, b5):
    """Host-side weight preprocessing + per-core input shards."""
    f = np.float32
    even = 2 * np.arange(C)

    def center(w):
        return (w - w.mean(axis=0, keepdims=True)).astype(f)

    sw = np.empty((C, 4 * C), f)   # lhsT layout [k, m], x2 for Dsqrt fold
    bw = np.empty((C, 4 * C), f)
    sb = np.empty((1, 4 * C), f)
    bb = np.empty((1, 4 * C), f)
    for L in range(4):
        rows = 256 * L + even
        sw[:, L * C:(L + 1) * C] = np.asarray(mw3)[rows, :].T
        bw[:, L * C:(L + 1) * C] = np.asarray(mw3)[rows + 1, :].T
        sb[0, L * C:(L + 1) * C] = np.asarray(mb3)[rows]
        bb[0, L * C:(L + 1) * C] = np.asarray(mb3)[rows + 1]

    wcs = [center(np.asarray(w)) for w in (w2, w3, w4)]
    wct = np.concatenate([w.T for w in wcs], axis=1).astype(f)  # [C, 3C]
    bct = np.concatenate([
        np.asarray(b) - np.asarray(b).mean() for b in (b1, b2, b3, b4)
    ]).reshape(1, 4 * C).astype(f)
    w1c = center(np.asarray(w1).reshape(C, 1))                  # [C,1]

    emit_sb = bool(np.any(sb) or np.any(bb))
    emit_tb = bool(np.any(bct))
    nw = 1857 + (1024 if emit_sb else 0) + (512 if emit_tb else 0)
    wpack = np.zeros((C, nw), f)
    wpack[0:MF, 0:64] = np.asarray(mw1).T
    wpack[0:64, 64:192] = np.asarray(mw2).T
    wpack[:, 192:704] = sw
    wpack[:, 704:1216] = bw
    wpack[:, 1216:1600] = wct
    wpack[:, 1600:1728] = np.full((C, C), 1.0 / (C - 1), f)
    wpack[0:1, 1728:1856] = w1c.T
    wpack[:, 1856:1857] = np.asarray(w5, f).reshape(1, C).T
    o = 1857
    if emit_sb:
        wpack[0:1, o:o + 512] = sb
        wpack[0:1, o + 512:o + 1024] = bb
        o += 1024
    if emit_tb:
        wpack[0:1, o:o + 512] = bct
    bpack = np.zeros((C, 4), f)
    bpack[0:64, 0] = np.asarray(mb1, f)
    bpack[:, 1] = np.asarray(mb2, f)
    bpack[0, 2] = float(np.asarray(b5).reshape(-1)[0])
    bpack[:, 3] = 1e-12
    shared = dict(wpack=wpack, bpack=bpack)
    xv = np.asarray(x, f).reshape(B)
    mdv = np.asarray(metadata, f)
    in_maps = []
    for c in range(NCORES):
        m = dict(shared)
        m["xt"] = np.ascontiguousarray(xv[c * BL:(c + 1) * BL].reshape(1, BL))
        m["mdt"] = np.ascontiguousarray(mdv[c * BL:(c + 1) * BL, :].T)
        in_maps.append(m)
    return in_maps, emit_sb, emit_tb, 0.0


def run(trace=False, reps=1, **inputs):
    in_maps, esb, etb, b5v = _prep(**inputs)
    nc = _build(esb, etb, b5v, reps=reps)
    res = bass_utils.run_bass_kernel_spmd(
        nc, in_maps, core_ids=list(range(NCORES)), trace=trace)
    out = np.concatenate([
        (np.asarray(res.results[c]["out"]).reshape(BL)
         / np.asarray(res.results[c]["den"]).reshape(BL))
        for c in range(NCORES)
    ]).reshape(B, 1).astype(np.float32)
    return out, res


def kernel(**inputs):
    out, _ = run(trace=False, **inputs)
    return out

